# revision 36
# baseline (speedup 1.0000x reference)
"""BatchChildSumTreeLSTM Trainium2 kernel.

Forest of T complete B-ary trees, processed leaves-first, sharded across
8 NeuronCores by tree (2 trees / core); the whole recursion is
device-local.  The kernel is ScalarE(ACT)-throughput bound (~218k
sigmoid/tanh columns at 1 col/cycle @1.2GHz), so everything else is
arranged to keep ACT saturated.

Per-core layout is feature-major: activations live in SBUF as
[128 features (partitions), n nodes (free)].  Matmuls are
out[H, n] = W.T.T @ rhs with pre-transposed weights as lhsT; PSUM
accumulates the x-projection with the h-recurrence, and gates are
evaluated by ScalarE straight out of PSUM with per-partition bias APs
in 2048-col groups (PSUM-capacity bound).

Level numbering (per core): n[l] = tpc*4**l, l = 0..7.  Levels 0..6 keep
the reference (interleaved) node order: children of node j at level l
are cols 4j..4j+3 of level l+1.  The LEAF level (7) is stored
plane-major: leaf col p*n6 + j is child #p of level-6 col j.  This makes
the leaf->6 child-sums contiguous accumulates (DVE 2x mode) and lets the
leaf f-gate reuse the level-6 x slice directly.  Upper-level f-gates
read the parent x through stride-0 (broadcast) matmul APs, so no
parent-broadcast input tensor is needed at all.

All elementwise tensors are bf16 (DVE 2x mode); PSUM / biases stay f32.
Upper-level x stays resident in SBUF (DMA'd in stages interleaved with
the leaf chunk stream); level-6 gate groups are computed one gate per
leaf iteration to smooth the PE burst.  h child-sums run on GpSimd
inside the leaf loop (VectorE headroom) but on VectorE in the drain,
where VectorE is idle and GpSimd latency would sit on the serial chain.
"""

import sys

if "/opt/trn_rl_repo" not in sys.path:
    sys.path.insert(0, "/opt/trn_rl_repo")

import numpy as np

P = 128          # feature dim == partitions
BR = 4           # branching factor
NLBL = 5

_NC_CACHE = {}


def _levels(tpc, depth):
    n = [tpc * BR**l for l in range(depth)]
    off = [0]
    for c in n:
        off.append(off[-1] + c)
    return n, off, off[-1]


def _legalize_waits(nc, max_waits=1):
    """This walrus build accepts at most one sync-wait command per
    instruction (any type).  Hoist excess waits onto same-engine NoOps
    inserted right before the instruction; engine program order makes
    this exactly equivalent."""
    import concourse.mybir as mybir

    n_nops = 0
    for fn in nc.m.functions:
        for blk in fn.blocks:
            new_insts = []
            for inst in blk.instructions:
                si = getattr(inst, "sync_info", None)
                if si is not None and si.on_wait and len(si.on_wait) > max_waits:
                    waits = list(si.on_wait)
                    hoist, keep = waits[:-max_waits], waits[-max_waits:]
                    eng = getattr(inst, "engine", None)
                    for j, w in enumerate(hoist):
                        nop = mybir.InstNoOp(
                            name=f"{inst.name}-wn{j}",
                            engine=eng,
                            bass_nofuse=True,
                        )
                        nop.sync_info = mybir.SyncInfo(on_wait=[w],
                                                       on_update=[])
                        new_insts.append(nop)
                        n_nops += 1
                    inst.sync_info = mybir.SyncInfo(
                        on_wait=keep, on_update=list(si.on_update))
                new_insts.append(inst)
            blk.instructions = new_insts
    return n_nops


def build_nc(tpc=2, depth=8, nlbl=NLBL, legalize=True):
    """Build the per-core Bass/Tile program (identical on all cores)."""
    import concourse.bass as bass
    import concourse.mybir as mybir
    import concourse.tile as tile

    f32 = mybir.dt.float32
    bf16 = mybir.dt.bfloat16
    AF = mybir.ActivationFunctionType
    ADD = mybir.AluOpType.add

    n, off, ntot = _levels(tpc, depth)
    lleaf = depth - 1           # 7
    l6 = depth - 2
    l5 = depth - 3
    l4 = depth - 4
    nleaf = n[lleaf]            # 32768
    n6 = n[l6]                  # 8192
    CH = 2048                   # leaf chunk / gate group size
    NR = n6 // CH               # number of level-6 parent ranges (4)
    PLANES = BR
    nup = off[lleaf]            # total cols in levels 0..6 (10922)

    nc = bass.Bass()

    xT = nc.dram_tensor("xT", [P, ntot], bf16, kind="ExternalInput")
    wnames = ["ix", "ih", "fx", "fh", "ox", "oh", "ux", "uh"]
    wall = nc.dram_tensor("Wall", [P, len(wnames), P], bf16,
                          kind="ExternalInput")
    bias4 = nc.dram_tensor("bias4", [P, 4], f32, kind="ExternalInput")
    woutT = nc.dram_tensor("WoutT", [P, nlbl], bf16, kind="ExternalInput")
    bout = nc.dram_tensor("bout2", [tpc, nlbl], f32, kind="ExternalInput")
    out = nc.dram_tensor("out", [tpc, nlbl], f32, kind="ExternalOutput")

    SIG, TANH = AF.Sigmoid, AF.Tanh
    BIDX = {"i": 0, "f": 1, "o": 2, "u": 3}

    with tile.TileContext(nc) as tc:
        import contextlib
        with contextlib.ExitStack() as ctx:
            wp = ctx.enter_context(tc.tile_pool(name="wp", bufs=1))
            xs = ctx.enter_context(tc.tile_pool(name="xs", bufs=3))
            hcs = ctx.enter_context(tc.tile_pool(name="hcs", bufs=3))
            chk = ctx.enter_context(tc.tile_pool(name="chk", bufs=2))
            tmp = ctx.enter_context(tc.tile_pool(name="tmp", bufs=2))
            acc = ctx.enter_context(tc.tile_pool(name="acc", bufs=1))
            hcp = ctx.enter_context(tc.tile_pool(name="hcp", bufs=1))
            psum = ctx.enter_context(
                tc.tile_pool(name="psum", bufs=2, space="PSUM"))

            def load_stream(tag, src, a, b, bufs=4):
                t = xs.tile([P, b - a], bf16, name=tag, tag=tag, bufs=bufs)
                nc.sync.dma_start(out=t, in_=src[:, a:b])
                return t

            def leaf_seg(r, p, lo=0, hi=CH):
                a = off[lleaf] + p * n6 + r * CH
                return a + lo, a + hi

            # ---- load constants + resident upper-level x (DMA order
            # matters: weights first, then the first leaf chunk so PE can
            # start ASAP, then the rest) ----
            wall_sb = wp.tile([P, len(wnames), P], bf16, name="wall_sb",
                              tag="wall_sb")
            # stagger the weight loads: leaf-gate weights (ix,ux,ox) first
            # so the first matmul isn't gated on the full 512KB transfer
            nc.sync.dma_start(out=wall_sb[:, 0, :], in_=wall[:, 0, :])
            wsb = {nm: wall_sb[:, j, :] for j, nm in enumerate(wnames)}
            segs = [(r, p, 0, CH) for r in range(NR) for p in range(PLANES)]
            a0, b0 = leaf_seg(*segs[0])
            nxt = xs.tile([P, b0 - a0], bf16, name="xt", tag="xt", bufs=4)
            nc.scalar.dma_start(out=nxt, in_=xT[:, a0:b0])
            bias_sb = wp.tile([P, 4], f32, name="bias_sb", tag="bias_sb")
            nc.sync.dma_start(out=bias_sb, in_=bias4[:])
            warm = wp.tile([P, 4], bf16, name="warm", tag="warm")
            nc.scalar.activation(out=warm, in_=bias_sb, func=SIG,
                                 bias=0.0, scale=1.0)
            for j in (6, 4):                  # ux, ox (leaf gates)
                nc.sync.dma_start(out=wall_sb[:, j, :], in_=wall[:, j, :])
            a1, b1 = leaf_seg(*segs[1])
            nxt2 = xs.tile([P, b1 - a1], bf16, name="xt", tag="xt", bufs=4)
            nc.scalar.dma_start(out=nxt2, in_=xT[:, a1:b1])
            # all of levels 0..6 x and parent-broadcast x stay resident,
            # but are DMA'd in small stages interleaved with the leaf
            # chunk stream (single HW queue) so chunks never starve
            xup = wp.tile([P, nup], bf16, name="xup", tag="xup")
            o6 = off[l6]
            o5 = off[l5]
            nc.scalar.dma_start(out=xup[:, o6:o6 + CH],
                                in_=xT[:, o6:o6 + CH])
            for j in (2, 3):                  # fx, fh (first leaf f-pass)
                nc.sync.dma_start(out=wall_sb[:, j, :], in_=wall[:, j, :])
            woutT_sb = wp.tile([P, nlbl], bf16, name="woutT_sb",
                               tag="woutT_sb")
            bout_sb = wp.tile([tpc, nlbl], f32, name="bout_sb", tag="bout_sb")

            # staged resident loads: iteration -> list of (dst, src, a, b)
            _W = ("w",)
            _stages = {
                0: [_W, (xup, xT, o6 + CH, o6 + 2 * CH)],
                1: [("wout",)],
                2: [(xup, xT, 0, o6)],
                6: [(xup, xT, o6 + 2 * CH, o6 + 3 * CH)],
                9: [(xup, xT, o6 + 3 * CH, o6 + 4 * CH)],
            }

            def run_stage(idx):
                for item in _stages.get(idx, ()):
                    if item == _W:
                        for j in (1, 5, 7):   # ih, oh, uh (level-6 gates)
                            nc.sync.dma_start(out=wall_sb[:, j, :],
                                              in_=wall[:, j, :])
                    elif item == ("wout",):
                        nc.sync.dma_start(out=woutT_sb, in_=woutT[:])
                        nc.sync.dma_start(out=bout_sb, in_=bout[:])
                    else:
                        dst, src, a, b = item
                        nc.sync.dma_start(out=dst[:, a:b], in_=src[:, a:b])

            def x6sl(r):
                return xup[:, off[l6] + r * CH:off[l6] + (r + 1) * CH]

            def bias_ap(g):
                i = BIDX[g]
                return bias_sb[:, i:i + 1]

            # persistent h/c for levels 0..5 (level 6 streams through chk)
            hres, cres = {}, {}
            for l in range(depth - 2):
                hres[l] = hcp.tile([P, n[l]], bf16, name=f"h{l}_sb",
                                   tag=f"h{l}_sb")
                cres[l] = hcp.tile([P, n[l]], bf16, name=f"c{l}_sb",
                                   tag=f"c{l}_sb")

            # level-6 child-sum accumulators (filled plane-by-plane)
            fcs6 = acc.tile([P, n6], bf16, name="fcs6", tag="fcs6")
            hs6 = acc.tile([P, n6], bf16, name="hs6", tag="hs6")
            # level-5 / level-4 child-sum accumulators
            fcs5 = acc.tile([P, n[l5]], bf16, name="fcs5", tag="fcs5")
            hs5 = acc.tile([P, n[l5]], bf16, name="hs5", tag="hs5")
            fcs4 = acc.tile([P, n[l4]], bf16, name="fcs4", tag="fcs4")
            hs4 = acc.tile([P, n[l4]], bf16, name="hs4", tag="hs4")

            def matmul_group(ps, w0, rhs0, w1=None, rhs1=None, G=CH,
                             bcast0=False):
                """ps[:, :G] = w0.T@rhs0 (+ w1.T@rhs1).  512-col banks.
                bcast0: rhs0 is the PARENT x slice [P, G//4]; each parent
                col is replicated to its 4 children via a stride-0 AP."""
                nb = (G + 511) // 512
                for b in range(nb):
                    s = b * 512
                    e = min(s + 512, G)
                    if bcast0:
                        r0 = rhs0[:, s // BR:e // BR].unsqueeze(2) \
                                 .broadcast_to([P, (e - s) // BR, BR])
                    else:
                        r0 = rhs0[:, s:e]
                    nc.tensor.matmul(ps[:, s:e], wsb[w0], r0,
                                     start=True, stop=(w1 is None))
                if w1 is not None:
                    for b in range(nb):
                        s = b * 512
                        e = min(s + 512, G)
                        nc.tensor.matmul(ps[:, s:e], wsb[w1], rhs1[:, s:e],
                                         start=False, stop=True)

            def gate_pass(xt_ap, hs_ap, fcs_ap, h_out, c_out, G):
                """Compute i,u,o gates + c,h for G parent columns."""
                leaf = hs_ap is None
                g_sb = {}
                for gname, wx, wh, func in (("i", "ix", "ih", SIG),
                                            ("u", "ux", "uh", TANH),
                                            ("o", "ox", "oh", SIG)):
                    ps = psum.tile([P, CH], f32, name=f"ps_{gname}",
                                   tag="ps")
                    if leaf:
                        matmul_group(ps, wx, xt_ap, G=G)
                    else:
                        matmul_group(ps, wx, xt_ap, wh, hs_ap, G=G)
                    g = tmp.tile([P, G], bf16, name=f"g_{gname}",
                                 tag=f"g_{gname}")
                    nc.scalar.activation(out=g, in_=ps[:, :G], func=func,
                                         bias=bias_ap(gname), scale=1.0)
                    g_sb[gname] = g
                nc.vector.tensor_mul(c_out, g_sb["i"], g_sb["u"])
                if fcs_ap is not None:
                    nc.vector.tensor_add(c_out, c_out, fcs_ap)
                tt = tmp.tile([P, G], bf16, name="tt", tag="tt")
                nc.scalar.activation(out=tt, in_=c_out, func=TANH,
                                     bias=0.0, scale=1.0)
                nc.vector.tensor_mul(h_out, g_sb["o"], tt)

            def f_sigmoid(xb_ap, hch_ap, S, bcast=False):
                """f = sigmoid(Wfx@xb + Wfh@hch + bfx) for S child cols.
                bcast: xb_ap is the parent x slice [P, S//4]."""
                ps = psum.tile([P, CH], f32, name="ps_f", tag="ps")
                matmul_group(ps, "fx", xb_ap, "fh", hch_ap, G=S,
                             bcast0=bcast)
                f = tmp.tile([P, S], bf16, name="g_f", tag="g_f")
                nc.scalar.activation(out=f, in_=ps[:, :S], func=SIG,
                                     bias=bias_ap("f"), scale=1.0)
                return f

            def f_pass_upper(xpar_ap, hch_ap, cch_ap, fcs_out, hs_out, S,
                             heng=None):
                """Upper-level f-pass: interleaved children, parent x
                broadcast via stride-0 matmul APs, pairwise strided
                reduces (x4) for fc on VectorE and h on heng (GpSimd
                in-loop, VectorE in the drain where it is idle)."""
                heng = heng or nc.gpsimd
                f = f_sigmoid(xpar_ap, hch_ap, S, bcast=True)
                fc = tmp.tile([P, S], bf16, name="fc", tag="fc")
                nc.vector.tensor_mul(fc, f, cch_ap)
                fcv = fc.rearrange("p (a two) -> p a two", two=2)
                rt = tmp.tile([P, S // 2], bf16, name="rt", tag="rt", bufs=1)
                nc.vector.tensor_add(rt, fcv[:, :, 0], fcv[:, :, 1])
                rtv = rt.rearrange("p (a two) -> p a two", two=2)
                nc.vector.tensor_add(fcs_out, rtv[:, :, 0], rtv[:, :, 1])
                hv = hch_ap.rearrange("p (a two) -> p a two", two=2)
                rt2 = tmp.tile([P, S // 2], bf16, name="rt2", tag="rt2", bufs=1)
                heng.tensor_tensor(rt2, hv[:, :, 0], hv[:, :, 1], ADD)
                rt2v = rt2.rearrange("p (a two) -> p a two", two=2)
                heng.tensor_tensor(hs_out, rt2v[:, :, 0], rt2v[:, :, 1],
                                   ADD)

            # ============== leaves fused with level 6 =====================
            # leaf chunks iterate (r, p): cols p*n6 + r*CH .. +CH, i.e.
            # child-plane p of level-6 parent range r.
            pending_l6g = None      # r ready for level-6 gates
            pending_l5f = None      # (h6o, c6o, r)
            pending_l5g = None      # a5 (parent col offset, 512 wide)
            pending_l4f = None      # a5

            def emit_leaf_f(h7p, c7p, r, p, lo, hi):
                """f-pass of one leaf chunk; accumulate into fcs6/hs6[r]."""
                G = hi - lo
                rs = slice(r * CH + lo, r * CH + hi)
                f = f_sigmoid(x6sl(r)[:, lo:hi], h7p, G)
                if p == 0:
                    nc.vector.tensor_mul(fcs6[:, rs], f, c7p)
                    nc.vector.tensor_scalar_mul(hs6[:, rs], h7p, 1.0)
                else:
                    fc = tmp.tile([P, G], bf16, name="fc", tag="fc")
                    nc.vector.tensor_mul(fc, f, c7p)
                    nc.vector.tensor_add(fcs6[:, rs], fcs6[:, rs], fc)
                    nc.vector.tensor_add(hs6[:, rs], hs6[:, rs], h7p)

            def emit_l6_gates_step(r, step, st):
                """One gate (i/u/o) of the level-6 gate group per
                iteration, so the PE burst spreads across the r-cycle
                instead of stalling ACT behind 8 back-to-back matmuls."""
                rs = slice(r * CH, (r + 1) * CH)
                if step == 0:
                    st = {"h6o": chk.tile([P, CH], bf16, name="h6o",
                                          tag="h6o"),
                          "c6o": chk.tile([P, CH], bf16, name="c6o",
                                          tag="c6o")}
                gname, wx, wh, func = (("i", "ix", "ih", SIG),
                                       ("u", "ux", "uh", TANH),
                                       ("o", "ox", "oh", SIG))[step]
                ps = psum.tile([P, CH], f32, name=f"ps6_{gname}", tag="ps")
                matmul_group(ps, wx, x6sl(r), wh, hs6[:, rs], G=CH)
                g = acc.tile([P, CH], bf16, name=f"g6_{gname}",
                             tag=f"g6_{gname}")
                nc.scalar.activation(out=g, in_=ps[:, :CH], func=func,
                                     bias=bias_ap(gname), scale=1.0)
                st[gname] = g
                if step == 2:
                    c6o, h6o = st["c6o"], st["h6o"]
                    nc.vector.tensor_mul(c6o, st["i"], st["u"])
                    nc.vector.tensor_add(c6o, c6o, fcs6[:, rs])
                    tt6 = tmp.tile([P, CH], bf16, name="tt", tag="tt")
                    nc.scalar.activation(out=tt6, in_=c6o, func=TANH,
                                         bias=0.0, scale=1.0)
                    nc.vector.tensor_mul(h6o, st["o"], tt6)
                return st

            def emit_l5_f(h6o, c6o, r, heng=None):
                # children: level-6 cols [r*CH, (r+1)*CH) -> l5 parents
                # [r*512, (r+1)*512)
                a5 = r * (CH // BR)
                q5 = CH // BR
                xp5 = xup[:, off[l5] + a5:off[l5] + a5 + q5]
                f_pass_upper(xp5, h6o, c6o, fcs5[:, a5:a5 + q5],
                             hs5[:, a5:a5 + q5], CH, heng=heng)

            def emit_l5_gates(a5):
                q5 = CH // BR
                xt5g = xup[:, off[l5] + a5:off[l5] + a5 + q5]
                gate_pass(xt5g, hs5[:, a5:a5 + q5], fcs5[:, a5:a5 + q5],
                          hres[l5][:, a5:a5 + q5], cres[l5][:, a5:a5 + q5],
                          q5)

            def emit_l4_f(a5, heng=None):
                q5 = CH // BR
                q4 = q5 // BR
                a4 = a5 // BR
                xp4 = xup[:, off[l4] + a4:off[l4] + a4 + q4]
                f_pass_upper(xp4, hres[l5][:, a5:a5 + q5],
                             cres[l5][:, a5:a5 + q5],
                             fcs4[:, a4:a4 + q4], hs4[:, a4:a4 + q4], q5,
                             heng=heng)

            # ---- main leaf loop ----
            prev = None     # (h7k, c7k, r, p, lo, hi) for deferred f-pass
            for idx, (r, p, lo, hi) in enumerate(segs):
                xt7 = nxt
                nxt = nxt2
                if idx + 2 < len(segs):
                    a, b = leaf_seg(*segs[idx + 2])
                    nxt2 = load_stream("xt", xT, a, b)
                run_stage(idx)
                G = hi - lo
                h7k = hcs.tile([P, G], bf16, name="hch", tag="hch")
                c7k = hcs.tile([P, G], bf16, name="cch", tag="cch")
                gate_pass(xt7, None, None, h7k, c7k, G)
                # previous chunk's f-pass FIRST so its sigmoid reaches the
                # ACT queue before the ladder's long matmul runs, then the
                # deferred ladder work from previous iterations
                lad = (pending_l4f, pending_l5g, pending_l5f, pending_l6g)
                pending_l4f = pending_l5g = pending_l5f = None
                pending_l6g = None
                if prev is not None:
                    emit_leaf_f(*prev)
                    if prev[3] == PLANES - 1 and prev[5] == CH:
                        pending_l6g = (prev[2], 0, None)
                d_l4f, d_l5g, d_l5f, d_l6g = lad
                if d_l4f is not None:
                    emit_l4_f(d_l4f)
                if d_l5g is not None:
                    emit_l5_gates(d_l5g)
                    pending_l4f = d_l5g
                if d_l5f is not None:
                    emit_l5_f(*d_l5f)
                    pending_l5g = d_l5f[2] * (CH // BR)
                if d_l6g is not None:
                    gr, step, st = d_l6g
                    st = emit_l6_gates_step(gr, step, st)
                    if step == 2:
                        pending_l5f = (st["h6o"], st["c6o"], gr)
                    else:
                        pending_l6g = (gr, step + 1, st)
                prev = (h7k, c7k, r, p, lo, hi)
            # drain the software pipeline
            if prev is not None:
                emit_leaf_f(*prev)
                if prev[3] == PLANES - 1 and prev[5] == CH:
                    pending_l6g = (prev[2], 0, None)
                prev = None
            while (pending_l6g is not None or pending_l5f is not None
                   or pending_l5g is not None or pending_l4f is not None):
                if pending_l4f is not None:
                    emit_l4_f(pending_l4f, heng=nc.vector)
                    pending_l4f = None
                if pending_l5g is not None:
                    emit_l5_gates(pending_l5g)
                    pending_l4f = pending_l5g
                    pending_l5g = None
                if pending_l5f is not None:
                    emit_l5_f(*pending_l5f, heng=nc.vector)
                    pending_l5g = pending_l5f[2] * (CH // BR)
                    pending_l5f = None
                if pending_l6g is not None:
                    gr, step, st = pending_l6g
                    st = emit_l6_gates_step(gr, step, st)
                    if step == 2:
                        pending_l5f = (st["h6o"], st["c6o"], gr)
                        pending_l6g = None
                    else:
                        pending_l6g = (gr, step + 1, st)

            # gates of level 4 (its child-sums are complete now)
            gate_pass(xup[:, off[l4]:off[l4] + n[l4]], hs4, fcs4,
                      hres[l4], cres[l4], n[l4])

            # ================= levels 3 .. 0 ==============================
            for l in range(depth - 5, -1, -1):
                npar, nch = n[l], n[l + 1]
                fcs = acc.tile([P, npar], bf16, name=f"fcs{l}", tag=f"fcs{l}")
                hs = acc.tile([P, npar], bf16, name=f"hs{l}", tag=f"hs{l}")
                f_pass_upper(xup[:, off[l]:off[l] + npar],
                             hres[l + 1], cres[l + 1], fcs, hs, nch,
                             heng=nc.vector)
                gate_pass(xup[:, off[l]:off[l] + npar], hs, fcs,
                          hres[l], cres[l], npar)

            # ================= output head ================================
            ps = psum.tile([P, CH], f32, name="ps_out", tag="ps")
            nc.tensor.matmul(ps[:tpc, :nlbl], hres[0], woutT_sb,
                             start=True, stop=True)
            out_sb = tmp.tile([tpc, nlbl], f32, name="out_sb", tag="out_sb")
            nc.vector.tensor_add(out_sb, ps[:tpc, :nlbl], bout_sb)
            nc.sync.dma_start(out=out[:], in_=out_sb)

    if legalize:
        _legalize_waits(nc)
    return nc


def _prep_core_inputs(embeds, weights, tpc, depth, n_cores):
    """Host-side shard + transpose per core.  Leaf level is permuted to
    plane-major (leaf col p*n6 + j = child #p of level-6 col j); levels
    0..6 keep reference order."""
    import ml_dtypes
    bf16 = ml_dtypes.bfloat16
    n, off, ntot = _levels(tpc, depth)
    T = tpc * n_cores
    counts = [T * BR**l for l in range(depth)]
    offsets = [0]
    for c in counts:
        offsets.append(offsets[-1] + c)
    lleaf = depth - 1
    n6 = n[depth - 2]

    common = dict(weights)
    in_maps = []
    for d in range(n_cores):
        shard = np.concatenate(
            [embeds[offsets[l] + tpc * d * BR**l:
                    offsets[l] + tpc * (d + 1) * BR**l]
             for l in range(depth)], axis=0)
        xT = np.ascontiguousarray(shard.T.astype(bf16))   # [P, ntot]
        # permute leaf level to plane-major: new col p*n6+j <- old col 4j+p
        leaf = xT[:, off[lleaf]:off[lleaf] + n[lleaf]]
        xT[:, off[lleaf]:] = np.ascontiguousarray(
            leaf.reshape(P, n6, BR).transpose(0, 2, 1).reshape(P, -1))
        m = {"xT": xT}
        m.update(common)
        in_maps.append(m)
    return in_maps


def _prep_weights(Wix, bix, Wih, Wfx, bfx, Wfh, Wox, box, Woh, Wux, bux, Wuh,
                  Wout, bout, tpc):
    import ml_dtypes
    f = np.float32
    bf = ml_dtypes.bfloat16
    # order must match build_nc's wnames: ix, ih, fx, fh, ox, oh, ux, uh
    wall = np.stack([Wix.T, Wih.T, Wfx.T, Wfh.T, Wox.T, Woh.T, Wux.T, Wuh.T],
                    axis=1)   # [128 (in-feat), 8, 128 (out-feat)]
    w = {
        "Wall": np.ascontiguousarray(wall, dtype=bf),
        "bias4": np.ascontiguousarray(
            np.stack([bix, bfx, box, bux], axis=1), dtype=f),
        "WoutT": np.ascontiguousarray(Wout.T, dtype=bf),
        "bout2": np.ascontiguousarray(np.tile(bout, (tpc, 1)), dtype=f),
    }
    return w


def _ensure_ntff_hook():
    """The RL container's antenv lacks axon_hooks; install a shim and
    register the ctypes NTFF profiler so trace=True works."""
    import types

    try:
        from antenv.axon_hooks import get_axon_ntff_profile_hook  # noqa
        return
    except ImportError:
        pass
    mod = types.ModuleType("antenv.axon_hooks")
    _h = [None]
    mod.set_axon_ntff_profile_hook = lambda h: _h.__setitem__(0, h)
    mod.get_axon_ntff_profile_hook = lambda: _h[0]
    sys.modules["antenv.axon_hooks"] = mod
    import antenv
    antenv.axon_hooks = mod
    try:
        from trn_agent_boot.trn_boot import _ntff_profile_via_ctypes
        h = _ntff_profile_via_ctypes("/opt/axon/libaxon_pjrt.so")
        if h is not None:
            mod.set_axon_ntff_profile_hook(h)
    except Exception:
        pass


def kernel(embeds, Wix, bix, Wih, Wfx, bfx, Wfh, Wox, box, Woh, Wux, bux, Wuh,
           Wout, bout, _trace=False):
    from concourse import bass_utils
    from concourse.bass_utils import run_bass_kernel_spmd

    if _trace:
        _ensure_ntff_hook()
        bass_utils.upload_artifacts = lambda d: d  # no S3 in this container

    n_cores = 8
    depth = 8
    T = 16
    tpc = T // n_cores

    embeds = np.asarray(embeds, dtype=np.float32)
    weights = _prep_weights(
        np.asarray(Wix), np.asarray(bix), np.asarray(Wih), np.asarray(Wfx),
        np.asarray(bfx), np.asarray(Wfh), np.asarray(Wox), np.asarray(box),
        np.asarray(Woh), np.asarray(Wux), np.asarray(bux), np.asarray(Wuh),
        np.asarray(Wout), np.asarray(bout), tpc)
    in_maps = _prep_core_inputs(embeds, weights, tpc, depth, n_cores)

    key = (tpc, depth)
    if key not in _NC_CACHE:
        _NC_CACHE[key] = build_nc(tpc=tpc, depth=depth)
    nc = _NC_CACHE[key]

    res = run_bass_kernel_spmd(nc, in_maps, core_ids=list(range(n_cores)),
                               trace=_trace)
    outs = np.concatenate([r["out"] for r in res.results], axis=0)
    if _trace:
        kernel.last_results = res
    return outs.astype(np.float32)


kernel.last_results = None
